# revision 13
# baseline (speedup 1.0000x reference)
"""Trainium2 Bass kernel for nn_Attn_3384434229614.

Reference computation:
    proj     = einsum('sbh,oh->sbo', encoder_outputs, W) + b    # [S,B,H]
    energies = einsum('bh,sbh->bs', hidden[0], proj)            # [B,S]
    attn     = softmax(energies, axis=1)[:, None, :]            # [B,1,S]

Algebraic rewrite (exact):
    energies[b,s] = enc[s,b,:] . v[b,:]  +  hidden[b,:] . bias
    with v = hidden[0] @ W.
The bias term is constant over s, so softmax is invariant to it and it is
dropped entirely. This turns a 137 GFLOP matmul into a 256 MiB streaming
dot-product reduction (memory bound).

Sharding: data-parallel over batch B=32 across 8 cores (4 batches/core);
W is replicated. Each core computes its own softmax (no collectives).
"""

import sys

import numpy as np

if "/opt/trn_rl_repo" not in sys.path:
    sys.path.insert(0, "/opt/trn_rl_repo")

S, B, H = 2048, 32, 1024
NCORES = 8
BL = B // NCORES          # 4 batches per core
PT = 128                  # s-tile partition size
NT = S // PT              # 16 s-tiles
KC = H // 128             # 8 contraction chunks for v = hidden @ W

_PROGRAM = None


def _build_program():
    """Build + compile the per-core Bass program (same on all 8 cores)."""
    import concourse.bass as bass  # noqa: F401  (registers engine classes)
    import concourse.bacc as bacc
    import concourse.mybir as mybir
    import concourse.tile as tile
    from concourse.masks import make_identity

    f32 = mybir.dt.float32
    Alu = mybir.AluOpType

    nc = bacc.Bacc("TRN2", target_bir_lowering=False, debug=False)

    enc = nc.dram_tensor("enc", [S, BL, H], f32, kind="ExternalInput").ap()
    hidT = nc.dram_tensor("hidT", [H, BL], f32, kind="ExternalInput").ap()
    w = nc.dram_tensor("w", [H, H], f32, kind="ExternalInput").ap()
    out = nc.dram_tensor("out", [BL, S], f32, kind="ExternalOutput").ap()
    # scratch for the [NT*BL, PT] -> [BL, S] partition rearrange
    e_dram = nc.dram_tensor("e_scratch", [NT * BL, PT], f32, kind="Internal").ap()

    with tile.TileContext(nc) as tc:
        with (
            tc.tile_pool(name="const", bufs=1) as constp,
            tc.tile_pool(name="wpool", bufs=1) as wp,
            tc.tile_pool(name="encp", bufs=8) as encp,
            tc.tile_pool(name="vflatp", bufs=2) as vfp,
            tc.tile_pool(name="smallp", bufs=1) as smallp,
            tc.tile_pool(name="psump", bufs=1, space="PSUM") as psp,
        ):
            # ---- preamble: v = hidden @ W, broadcast across partitions ----
            # hidT first (tiny), then W per k-chunk so the PE matmuls start
            # as soon as each chunk lands instead of after the full 4 MiB.
            hid_sb = constp.tile([128, KC, BL], f32)
            nc.sync.dma_start(hid_sb[:], hidT.rearrange("(c p) b -> p c b", p=128))
            # W lives in two enc-pool slots (same shape/tag as enc tiles) so
            # its SBUF is recycled for enc prefetch once the matmuls consume it
            wr = w.rearrange("(c p) h -> p c h", p=128)
            w_halves = []
            for half in range(2):
                wt = encp.tile([128, BL, H], f32, tag="et")
                for cc in range(KC // 2):
                    c = half * (KC // 2) + cc
                    nc.sync.dma_start(wt[:, cc, :], wr[:, c, :])
                w_halves.append(wt)

            def w_chunk(c):
                return w_halves[c // (KC // 2)][:, c % (KC // 2), :]

            # preload the Exp activation table while everything else runs
            dummy = constp.tile([1, 1], f32)
            nc.gpsimd.memset(dummy[:], 0.0)
            nc.scalar.activation(
                dummy[:], dummy[:], mybir.ActivationFunctionType.Exp
            )

            # identity (also used for PE warm-up matmuls below)
            ident = constp.tile([128, 128], f32)
            make_identity(nc, ident[:])

            # warm the PE p-state with junk matmuls so the fp32 v-matmuls
            # below run at full clock instead of the cold 1.2 GHz state
            warm_src = constp.tile([128, 512], f32)
            nc.gpsimd.memset(warm_src[:], 0.0)
            psum_warm = psp.tile([128, 512], f32)
            for _ in range(2):
                nc.tensor.matmul(
                    psum_warm[:], ident[:], warm_src[:], start=True, stop=True
                )

            psum_v = psp.tile([BL, H], f32)
            for c in range(KC):
                for n in range(H // 512):
                    nc.tensor.matmul(
                        psum_v[:, n * 512 : (n + 1) * 512],
                        hid_sb[:, c, :],
                        w_chunk(c)[:, n * 512 : (n + 1) * 512],
                        start=(c == 0),
                        stop=(c == KC - 1),
                    )
            v_sb = smallp.tile([BL, H], f32)
            nc.scalar.copy(v_sb[:], psum_v[:])

            # fold each v row into partition 0, broadcast to all 128 per
            # batch so the first DVE op starts before all rows are done
            v_rep = wp.tile([128, BL, H], f32)
            for bb in range(BL):
                v_flat = vfp.tile([1, H], f32)
                nc.sync.dma_start(v_flat[:], v_sb[bb : bb + 1, :])
                nc.gpsimd.partition_broadcast(v_rep[:, bb, :], v_flat[:])

            # ---- main loop: energies via fused multiply+row-sum on DVE ----
            # The product tensor is written in-place into the enc tile (it is
            # never read); accum_out collects the per-row dot products.
            e_sb = smallp.tile([128, NT * BL], f32)

            def stt(et, bb, col):
                nc.vector.scalar_tensor_tensor(
                    out=et[:, bb, :],
                    in0=et[:, bb, :],
                    scalar=1.0,
                    in1=v_rep[:, bb, :],
                    op0=Alu.mult,
                    op1=Alu.mult,
                    accum_out=e_sb[:, col : col + 1],
                )

            for st in range(NT):
                et = encp.tile([128, BL, H], f32, tag="et")
                if st < NT - 1:
                    nc.sync.dma_start(et[:], enc[st * PT : (st + 1) * PT])
                    for bb in range(BL):
                        stt(et, bb, st * BL + bb)
                else:
                    # split the last tile per batch so the trailing DVE op
                    # starts as soon as its quarter lands
                    for bb in range(BL):
                        nc.sync.dma_start(
                            et[:, bb, :], enc[st * PT : (st + 1) * PT, bb, :]
                        )
                        stt(et, bb, st * BL + bb)

            # ---- transpose energies to [BL, S] layout ----
            psum_t = psp.tile([NT * BL, 128], f32)
            nc.tensor.transpose(psum_t[:], e_sb[:], ident[:])
            e_t = smallp.tile([NT * BL, 128], f32)
            nc.scalar.copy(e_t[:], psum_t[:])
            nc.sync.dma_start(e_dram[:], e_t[:])
            ebs = smallp.tile([BL, S], f32)
            nc.sync.dma_start(
                ebs[:].rearrange("b (t p) -> b t p", t=NT),
                e_dram.rearrange("(t b) p -> b t p", b=BL),
            )

            # ---- softmax over free axis (per-partition batch rows) ----
            nmx = smallp.tile([BL, 1], f32)
            nc.vector.reduce_max(
                nmx[:], ebs[:], axis=mybir.AxisListType.X, negate=True
            )
            ex = smallp.tile([BL, S], f32)
            sm = smallp.tile([BL, 1], f32)
            nc.scalar.activation(
                ex[:],
                ebs[:],
                mybir.ActivationFunctionType.Exp,
                bias=nmx[:],
                scale=1.0,
                accum_out=sm[:],
            )
            rs = smallp.tile([BL, 1], f32)
            nc.vector.reciprocal(rs[:], sm[:])
            nc.vector.tensor_scalar_mul(ebs[:], ex[:], rs[:])
            nc.sync.dma_start(out[:], ebs[:])

    nc.compile()
    return nc


def _get_program():
    global _PROGRAM
    if _PROGRAM is None:
        _PROGRAM = _build_program()
    return _PROGRAM


def make_in_maps(hidden, encoder_outputs, W):
    hidden = np.asarray(hidden, dtype=np.float32)
    encoder_outputs = np.asarray(encoder_outputs, dtype=np.float32)
    W = np.ascontiguousarray(np.asarray(W, dtype=np.float32))
    in_maps = []
    for m in range(NCORES):
        sl = slice(m * BL, (m + 1) * BL)
        in_maps.append(
            {
                "enc": np.ascontiguousarray(encoder_outputs[:, sl, :]),
                "hidT": np.ascontiguousarray(hidden[0, sl, :].T),
                "w": W,
            }
        )
    return in_maps


def run_sharded(hidden, encoder_outputs, W, **spmd_kwargs):
    """Run the SPMD kernel on all 8 cores; returns BassKernelResults."""
    from concourse import bass_utils

    nc = _get_program()
    in_maps = make_in_maps(hidden, encoder_outputs, W)
    return bass_utils.run_bass_kernel_spmd(
        nc, in_maps, core_ids=list(range(NCORES)), **spmd_kwargs
    )


def kernel(hidden, encoder_outputs, W, b):
    # b only shifts every energy of a batch row by the same constant
    # (hidden[b,:] . bias), which softmax cancels exactly -> unused.
    res = run_sharded(hidden, encoder_outputs, W)
    attn = np.concatenate([r["out"] for r in res.results], axis=0)  # [B, S]
    return attn[:, None, :].astype(np.float32)


# revision 27
# speedup vs baseline: 881.3862x; 881.3862x over previous
"""Trainium2 Bass kernel for nn_Attn_3384434229614.

Reference computation:
    proj     = einsum('sbh,oh->sbo', encoder_outputs, W) + b    # [S,B,H]
    energies = einsum('bh,sbh->bs', hidden[0], proj)            # [B,S]
    attn     = softmax(energies, axis=1)[:, None, :]            # [B,1,S]

Algebraic rewrite (exact):
    energies[b,s] = enc[s,b,:] . v[b,:]  +  hidden[b,:] . bias
    with v = hidden[0] @ W.
The bias term is constant over s, so softmax is invariant to it and it is
dropped entirely. This turns a 137 GFLOP matmul into a 256 MiB streaming
dot-product reduction (memory bound).

Sharding: data-parallel over batch B=32 across 8 cores (4 batches/core);
W is replicated. Each core computes its own softmax (no collectives).
"""

import sys

import numpy as np

if "/opt/trn_rl_repo" not in sys.path:
    sys.path.insert(0, "/opt/trn_rl_repo")

S, B, H = 2048, 32, 1024
NCORES = 8
BL = B // NCORES          # 4 batches per core
PT = 128                  # s-tile partition size
NT = S // PT              # 16 s-tiles
KC = H // 128             # 8 contraction chunks for v = hidden @ W

_PROGRAM = None


def _build_program(repeat=1):
    """Build + compile the per-core Bass program (same on all 8 cores)."""
    import concourse.bass as bass  # noqa: F401  (registers engine classes)
    import concourse.bacc as bacc
    import concourse.mybir as mybir
    import concourse.tile as tile
    from concourse.masks import make_identity

    f32 = mybir.dt.float32
    Alu = mybir.AluOpType

    nc = bacc.Bacc("TRN2", target_bir_lowering=False, debug=False)

    enc = nc.dram_tensor("enc", [S, BL, H], f32, kind="ExternalInput").ap()
    hidT = nc.dram_tensor("hidT", [H, BL], f32, kind="ExternalInput").ap()
    w = nc.dram_tensor("w", [H, H], f32, kind="ExternalInput").ap()
    out = nc.dram_tensor("out", [BL, S], f32, kind="ExternalOutput").ap()

    with tile.TileContext(nc) as tc:
        with (
            tc.tile_pool(name="const", bufs=1) as constp,
            tc.tile_pool(name="wpool", bufs=1) as wp,
            tc.tile_pool(name="encp", bufs=9) as encp,
            tc.tile_pool(name="vflatp", bufs=2) as vfp,
            tc.tile_pool(name="smallp", bufs=1) as smallp,
            tc.tile_pool(name="psump", bufs=1, space="PSUM") as psp,
            tc.tile_pool(name="dramp", bufs=1, space="DRAM") as drp,
        ):
            # DRAM scratch as pool tiles so Tile tracks the write->read deps
            # of the partition-rearrange round-trips
            e_dram = drp.tile([NT * BL, PT], f32)
            nm_dram = drp.tile([NT * BL, 1], f32)
            # ---- preamble: v = hidden @ W, broadcast across partitions ----
            # hidT first (tiny), then W per k-chunk so the PE matmuls start
            # as soon as each chunk lands instead of after the full 4 MiB.
            hid_sb = constp.tile([128, KC, BL], f32)
            nc.scalar.dma_start(hid_sb[:], hidT.rearrange("(c p) b -> p c b", p=128))
            # W lives in two enc-pool slots (same shape/tag as enc tiles) so
            # its SBUF is recycled for enc prefetch once the matmuls consume it
            wr = w.rearrange("(c p) h -> p c h", p=128)
            w_halves = []
            for half in range(2):
                wt = encp.tile([128, BL, H], f32, tag="et")
                for cc in range(KC // 2):
                    c = half * (KC // 2) + cc
                    nc.sync.dma_start(wt[:, cc, :], wr[:, c, :])
                w_halves.append(wt)

            def w_chunk(c):
                return w_halves[c // (KC // 2)][:, c % (KC // 2), :]

            # preload the Exp activation table while everything else runs
            dummy = constp.tile([1, 1], f32)
            nc.gpsimd.memset(dummy[:], 0.0)
            nc.scalar.activation(
                dummy[:], dummy[:], mybir.ActivationFunctionType.Exp
            )

            # identity (also used for PE warm-up matmuls below)
            ident = constp.tile([128, 128], f32)
            make_identity(nc, ident[:])

            # warm the PE p-state with junk matmuls so the fp32 v-matmuls
            # below run at full clock instead of the cold 1.2 GHz state
            warm_src = constp.tile([128, 512], f32)
            nc.gpsimd.memset(warm_src[:], 0.0)
            psum_warm = psp.tile([128, 512], f32)
            for _ in range(2):
                nc.tensor.matmul(
                    psum_warm[:], ident[:], warm_src[:], start=True, stop=True
                )

            psum_v = psp.tile([BL, H], f32)
            for c in range(KC):
                for n in range(H // 512):
                    nc.tensor.matmul(
                        psum_v[:, n * 512 : (n + 1) * 512],
                        hid_sb[:, c, :],
                        w_chunk(c)[:, n * 512 : (n + 1) * 512],
                        start=(c == 0),
                        stop=(c == KC - 1),
                    )
            v_sb = smallp.tile([BL, H], f32)
            nc.scalar.copy(v_sb[:], psum_v[:])

            # fold each v row into partition 0, broadcast to all 128 per
            # batch so the first DVE op starts before all rows are done
            v_rep = wp.tile([128, BL, H], f32)
            for bb in range(BL):
                v_flat = vfp.tile([1, H], f32)
                nc.sync.dma_start(v_flat[:], v_sb[bb : bb + 1, :])
                nc.gpsimd.partition_broadcast(v_rep[:, bb, :], v_flat[:])

            # ---- main loop: energies via fused multiply+row-sum on DVE ----
            # The product tensor is written in-place into the enc tile (it is
            # never read); accum_out collects the per-row dot products.
            e_sb = smallp.tile([128, NT * BL], f32)

            def stt(et, bb, col):
                nc.vector.scalar_tensor_tensor(
                    out=et[:, bb, :],
                    in0=et[:, bb, :],
                    scalar=1.0,
                    in1=v_rep[:, bb, :],
                    op0=Alu.mult,
                    op1=Alu.mult,
                    accum_out=e_sb[:, col : col + 1],
                )

            for _rep in range(repeat):
                for st in range(NT):
                    et = encp.tile([128, BL, H], f32, tag="et")
                    if st < NT - 3 or _rep < repeat - 1:
                        nc.sync.dma_start(et[:], enc[st * PT : (st + 1) * PT])
                        for bb in range(BL):
                            stt(et, bb, bb * NT + st)
                    else:
                        # split the last three tiles per batch so the trailing
                        # DVE ops start as soon as each quarter lands
                        for bb in range(BL):
                            nc.sync.dma_start(
                                et[:, bb, :], enc[st * PT : (st + 1) * PT, bb, :]
                            )
                            stt(et, bb, bb * NT + st)

            # ---- transpose energies to [BL, S] layout ----
            psum_t = psp.tile([NT * BL, 128], f32)
            nc.tensor.transpose(psum_t[:], e_sb[:], ident[:])
            e_t = smallp.tile([NT * BL, 128], f32)
            nc.scalar.copy(e_t[:], psum_t[:])
            nc.sync.dma_start(e_dram[:], e_t[:])
            ebs = smallp.tile([BL, S], f32)
            nc.sync.dma_start(
                ebs[:].rearrange("b (t p) -> b t p", t=NT),
                e_dram[:].rearrange("(b t) p -> b t p", b=BL),
            )

            # row maxes in the [64, 128] layout; their fold to [BL, 16] rides
            # a separate DMA queue, hidden under the big rearrange round-trip
            nm1 = smallp.tile([NT * BL, 1], f32)
            nc.vector.reduce_max(
                nm1[:], e_t[:], axis=mybir.AxisListType.X, negate=True
            )
            nc.scalar.dma_start(nm_dram[:], nm1[:])
            nm16 = smallp.tile([BL, NT], f32)
            nc.scalar.dma_start(
                nm16[:].rearrange("b (t o) -> b t o", t=NT),
                nm_dram[:].rearrange("(b t) o -> b t o", b=BL),
            )

            # ---- softmax over free axis (per-partition batch rows) ----
            nmx = smallp.tile([BL, 1], f32)
            nc.vector.tensor_reduce(
                nmx[:], nm16[:], axis=mybir.AxisListType.X, op=Alu.min
            )
            ex = smallp.tile([BL, S], f32)
            sm = smallp.tile([BL, 1], f32)
            nc.scalar.activation(
                ex[:],
                ebs[:],
                mybir.ActivationFunctionType.Exp,
                bias=nmx[:],
                scale=1.0,
                accum_out=sm[:],
            )
            rs = smallp.tile([BL, 1], f32)
            nc.vector.reciprocal(rs[:], sm[:])
            nc.vector.tensor_scalar_mul(ebs[:], ex[:], rs[:])
            nc.sync.dma_start(out[:], ebs[:])

    nc.compile()
    return nc


def _get_program():
    global _PROGRAM
    if _PROGRAM is None:
        _PROGRAM = _build_program()
    return _PROGRAM


def make_in_maps(hidden, encoder_outputs, W):
    hidden = np.asarray(hidden, dtype=np.float32)
    encoder_outputs = np.asarray(encoder_outputs, dtype=np.float32)
    W = np.ascontiguousarray(np.asarray(W, dtype=np.float32))
    in_maps = []
    for m in range(NCORES):
        sl = slice(m * BL, (m + 1) * BL)
        in_maps.append(
            {
                "enc": np.ascontiguousarray(encoder_outputs[:, sl, :]),
                "hidT": np.ascontiguousarray(hidden[0, sl, :].T),
                "w": W,
            }
        )
    return in_maps


def run_sharded(hidden, encoder_outputs, W, **spmd_kwargs):
    """Run the SPMD kernel on all 8 cores; returns BassKernelResults."""
    from concourse import bass_utils

    nc = _get_program()
    in_maps = make_in_maps(hidden, encoder_outputs, W)
    return bass_utils.run_bass_kernel_spmd(
        nc, in_maps, core_ids=list(range(NCORES)), **spmd_kwargs
    )


def kernel(hidden, encoder_outputs, W, b):
    # b only shifts every energy of a batch row by the same constant
    # (hidden[b,:] . bias), which softmax cancels exactly -> unused.
    res = run_sharded(hidden, encoder_outputs, W)
    attn = np.concatenate([r["out"] for r in res.results], axis=0)  # [B, S]
    return attn[:, None, :].astype(np.float32)


# revision 28
# speedup vs baseline: 882.8890x; 1.0017x over previous
"""Trainium2 Bass kernel for nn_Attn_3384434229614.

Reference computation:
    proj     = einsum('sbh,oh->sbo', encoder_outputs, W) + b    # [S,B,H]
    energies = einsum('bh,sbh->bs', hidden[0], proj)            # [B,S]
    attn     = softmax(energies, axis=1)[:, None, :]            # [B,1,S]

Algebraic rewrite (exact):
    energies[b,s] = enc[s,b,:] . v[b,:]  +  hidden[b,:] . bias
    with v = hidden[0] @ W.
The bias term is constant over s, so softmax is invariant to it and it is
dropped entirely. This turns a 137 GFLOP matmul into a 256 MiB streaming
dot-product reduction (memory bound).

Sharding: data-parallel over batch B=32 across 8 cores (4 batches/core);
W is replicated. Each core computes its own softmax (no collectives).
"""

import sys

import numpy as np

if "/opt/trn_rl_repo" not in sys.path:
    sys.path.insert(0, "/opt/trn_rl_repo")

S, B, H = 2048, 32, 1024
NCORES = 8
BL = B // NCORES          # 4 batches per core
PT = 128                  # s-tile partition size
NT = S // PT              # 16 s-tiles
KC = H // 128             # 8 contraction chunks for v = hidden @ W

_PROGRAM = None


def _build_program(repeat=1):
    """Build + compile the per-core Bass program (same on all 8 cores)."""
    import concourse.bass as bass  # noqa: F401  (registers engine classes)
    import concourse.bacc as bacc
    import concourse.mybir as mybir
    import concourse.tile as tile
    from concourse.masks import make_identity

    f32 = mybir.dt.float32
    Alu = mybir.AluOpType

    nc = bacc.Bacc("TRN2", target_bir_lowering=False, debug=False)

    enc = nc.dram_tensor("enc", [S, BL, H], f32, kind="ExternalInput").ap()
    hidT = nc.dram_tensor("hidT", [H, BL], f32, kind="ExternalInput").ap()
    w = nc.dram_tensor("w", [H, H], f32, kind="ExternalInput").ap()
    out = nc.dram_tensor("out", [BL, S], f32, kind="ExternalOutput").ap()

    with tile.TileContext(nc) as tc:
        with (
            tc.tile_pool(name="const", bufs=1) as constp,
            tc.tile_pool(name="wpool", bufs=1) as wp,
            tc.tile_pool(name="encp", bufs=9) as encp,
            tc.tile_pool(name="vflatp", bufs=2) as vfp,
            tc.tile_pool(name="smallp", bufs=1) as smallp,
            tc.tile_pool(name="psump", bufs=1, space="PSUM") as psp,
            tc.tile_pool(name="dramp", bufs=1, space="DRAM") as drp,
        ):
            # DRAM scratch as pool tiles so Tile tracks the write->read deps
            # of the partition-rearrange round-trips
            e_dram = drp.tile([NT * BL, PT], f32)
            nm_dram = drp.tile([NT * BL, 1], f32)
            # ---- preamble: v = hidden @ W, broadcast across partitions ----
            # hidT first (tiny), then W per k-chunk so the PE matmuls start
            # as soon as each chunk lands instead of after the full 4 MiB.
            hid_sb = constp.tile([128, KC, BL], f32)
            nc.scalar.dma_start(hid_sb[:], hidT.rearrange("(c p) b -> p c b", p=128))
            # W lives in two enc-pool slots (same shape/tag as enc tiles) so
            # its SBUF is recycled for enc prefetch once the matmuls consume it
            wr = w.rearrange("(c p) h -> p c h", p=128)
            w_halves = []
            for half in range(2):
                wt = encp.tile([128, BL, H], f32, tag="et")
                for cc in range(KC // 2):
                    c = half * (KC // 2) + cc
                    nc.sync.dma_start(wt[:, cc, :], wr[:, c, :])
                w_halves.append(wt)

            def w_chunk(c):
                return w_halves[c // (KC // 2)][:, c % (KC // 2), :]

            # preload the Exp activation table while everything else runs
            dummy = constp.tile([1, 1], f32)
            nc.gpsimd.memset(dummy[:], 0.0)
            nc.scalar.activation(
                dummy[:], dummy[:], mybir.ActivationFunctionType.Exp
            )

            # identity (also used for PE warm-up matmuls below)
            ident = constp.tile([128, 128], f32)
            make_identity(nc, ident[:])

            # warm the PE p-state with junk matmuls so the fp32 v-matmuls
            # below run at full clock instead of the cold 1.2 GHz state
            warm_src = constp.tile([128, 512], f32)
            nc.gpsimd.memset(warm_src[:], 0.0)
            psum_warm = psp.tile([128, 512], f32)
            for _ in range(2):
                nc.tensor.matmul(
                    psum_warm[:], ident[:], warm_src[:], start=True, stop=True
                )

            psum_v = psp.tile([BL, H], f32)
            for c in range(KC):
                for n in range(H // 512):
                    nc.tensor.matmul(
                        psum_v[:, n * 512 : (n + 1) * 512],
                        hid_sb[:, c, :],
                        w_chunk(c)[:, n * 512 : (n + 1) * 512],
                        start=(c == 0),
                        stop=(c == KC - 1),
                    )
            v_sb = smallp.tile([BL, H], f32)
            nc.scalar.copy(v_sb[:], psum_v[:])

            # fold each v row into partition 0, broadcast to all 128 per
            # batch so the first DVE op starts before all rows are done
            v_rep = wp.tile([128, BL, H], f32)
            for bb in range(BL):
                v_flat = vfp.tile([1, H], f32)
                nc.sync.dma_start(v_flat[:], v_sb[bb : bb + 1, :])
                nc.gpsimd.partition_broadcast(v_rep[:, bb, :], v_flat[:])

            # ---- main loop: energies via fused multiply+row-sum on DVE ----
            # The product tensor is written in-place into the enc tile (it is
            # never read); accum_out collects the per-row dot products.
            e_sb = smallp.tile([128, NT * BL], f32)

            def stt(et, bb, col):
                nc.vector.scalar_tensor_tensor(
                    out=et[:, bb, :],
                    in0=et[:, bb, :],
                    scalar=1.0,
                    in1=v_rep[:, bb, :],
                    op0=Alu.mult,
                    op1=Alu.mult,
                    accum_out=e_sb[:, col : col + 1],
                )

            for _rep in range(repeat):
                for st in range(NT):
                    et = encp.tile([128, BL, H], f32, tag="et")
                    if st < NT - 4 or _rep < repeat - 1:
                        nc.sync.dma_start(et[:], enc[st * PT : (st + 1) * PT])
                        for bb in range(BL):
                            stt(et, bb, bb * NT + st)
                    else:
                        # split the last four tiles per batch so the trailing
                        # DVE ops start as soon as each quarter lands
                        for bb in range(BL):
                            nc.sync.dma_start(
                                et[:, bb, :], enc[st * PT : (st + 1) * PT, bb, :]
                            )
                            stt(et, bb, bb * NT + st)

            # ---- transpose energies to [BL, S] layout ----
            psum_t = psp.tile([NT * BL, 128], f32)
            nc.tensor.transpose(psum_t[:], e_sb[:], ident[:])
            e_t = smallp.tile([NT * BL, 128], f32)
            nc.scalar.copy(e_t[:], psum_t[:])
            nc.sync.dma_start(e_dram[:], e_t[:])
            ebs = smallp.tile([BL, S], f32)
            nc.sync.dma_start(
                ebs[:].rearrange("b (t p) -> b t p", t=NT),
                e_dram[:].rearrange("(b t) p -> b t p", b=BL),
            )

            # row maxes in the [64, 128] layout; their fold to [BL, 16] rides
            # a separate DMA queue, hidden under the big rearrange round-trip
            nm1 = smallp.tile([NT * BL, 1], f32)
            nc.vector.reduce_max(
                nm1[:], e_t[:], axis=mybir.AxisListType.X, negate=True
            )
            nc.scalar.dma_start(nm_dram[:], nm1[:])
            nm16 = smallp.tile([BL, NT], f32)
            nc.scalar.dma_start(
                nm16[:].rearrange("b (t o) -> b t o", t=NT),
                nm_dram[:].rearrange("(b t) o -> b t o", b=BL),
            )

            # ---- softmax over free axis (per-partition batch rows) ----
            nmx = smallp.tile([BL, 1], f32)
            nc.vector.tensor_reduce(
                nmx[:], nm16[:], axis=mybir.AxisListType.X, op=Alu.min
            )
            ex = smallp.tile([BL, S], f32)
            sm = smallp.tile([BL, 1], f32)
            nc.scalar.activation(
                ex[:],
                ebs[:],
                mybir.ActivationFunctionType.Exp,
                bias=nmx[:],
                scale=1.0,
                accum_out=sm[:],
            )
            rs = smallp.tile([BL, 1], f32)
            nc.vector.reciprocal(rs[:], sm[:])
            nc.vector.tensor_scalar_mul(ebs[:], ex[:], rs[:])
            nc.sync.dma_start(out[:], ebs[:])

    nc.compile()
    return nc


def _get_program():
    global _PROGRAM
    if _PROGRAM is None:
        _PROGRAM = _build_program()
    return _PROGRAM


def make_in_maps(hidden, encoder_outputs, W):
    hidden = np.asarray(hidden, dtype=np.float32)
    encoder_outputs = np.asarray(encoder_outputs, dtype=np.float32)
    W = np.ascontiguousarray(np.asarray(W, dtype=np.float32))
    in_maps = []
    for m in range(NCORES):
        sl = slice(m * BL, (m + 1) * BL)
        in_maps.append(
            {
                "enc": np.ascontiguousarray(encoder_outputs[:, sl, :]),
                "hidT": np.ascontiguousarray(hidden[0, sl, :].T),
                "w": W,
            }
        )
    return in_maps


def run_sharded(hidden, encoder_outputs, W, **spmd_kwargs):
    """Run the SPMD kernel on all 8 cores; returns BassKernelResults."""
    from concourse import bass_utils

    nc = _get_program()
    in_maps = make_in_maps(hidden, encoder_outputs, W)
    return bass_utils.run_bass_kernel_spmd(
        nc, in_maps, core_ids=list(range(NCORES)), **spmd_kwargs
    )


def kernel(hidden, encoder_outputs, W, b):
    # b only shifts every energy of a batch row by the same constant
    # (hidden[b,:] . bias), which softmax cancels exactly -> unused.
    res = run_sharded(hidden, encoder_outputs, W)
    attn = np.concatenate([r["out"] for r in res.results], axis=0)  # [B, S]
    return attn[:, None, :].astype(np.float32)
